# revision 31
# baseline (speedup 1.0000x reference)
"""BatchHardTripletLoss kernel for 8 Trainium2 NeuronCores.

Math (matches the jax reference):
  dist2[i,j] = |e1_i|^2 + |e2_j|^2 - 2 e1.e2 + 2*eps*(s1_i - s2_j) + D*eps^2
             = a[i] + v[i,j],   v[i,j] = b[j] - 2<e1_i, e2_j>
  pos_max[i] = sqrt(clip(a[i] + max_{j in pos} v[i,j], 0))
  neg_min[i] = sqrt(clip(a[i] + min_{j in neg} v[i,j], 0))
  loss = mean over POS anchors of relu(pos_max - neg_min + margin)

Key structural points (vs the 106.7us first-cut kernel):
  * Only rows with target[i]==1 are computed (the loss ignores the
    rest): k//1024*1024 rows on device, the remainder exactly on host.
  * Neg-class e2 columns (and their bias) are sign-flipped so both
    classes are MAX reductions.
  * Mains run in fp8 (e4m3, K=128, all dims): the PE streams 2
    columns/cycle, so a 512-col matmul issues every ~216ns even at
    the cold (K=4/8 HAM) clock.  K=2 bf16 tails add the bias in
    bf16 hi+lo.  Verified end-to-end fp8 rel err ~6.8e-4 (tol 2e-2).
  * The bias TAILS run FIRST (start=True) and the mains LAST: the
    drain of a group unblocks as soon as its mains finish, which
    overlaps the scheduler's weight-batched [tails A,B][mains A,B]
    PE order instead of serializing PE-phase -> drain-phase.
  * Drain: the Act engine copies each PSUM group to SBUF bf16
    (1 elem/cyc); the DVE folds the copies in-place into a
    [128,1024] bf16 accumulator per (i-tile, class) with stock
    TENSOR_TENSOR max ops at the packed 2x rate, then one 1x
    TENSOR_SCALAR accumulate-max per (i-tile, class) reduces the
    accumulator into the output, chained with any direct-reduced
    boundary slivers via its per-partition scalar operand.

Host: pos-first column sort, f64 row stats, bf16 hi/lo bias split,
transposes, fp8/bf16 casts, final sqrt/margin/mean + exact f64
remainder rows.
"""

import os
import sys

for _p in ("/opt/trn_rl_repo",):
    if _p not in sys.path:
        sys.path.insert(0, _p)

import numpy as np
import ml_dtypes

EPS = 1e-6
MARGIN = 0.2
B = 8192
D = 128
NCORES = 8
GW = 2048             # candidate-group width = 4 PSUM banks
NG = B // GW
PSUM_BUFS = 2
BIG = 1.0e30
MIN_COPY = 512        # segments narrower than this reduce directly from PSUM
ACCW = 1024           # bf16 accumulator width per (i-tile, class)

_programs = {}
LAST_RESULTS = None   # BassKernelResults of the most recent run (for profiling)


def _build_program(n_it: int, k: int):
    """Bass program for one core.

    n_it: i-tiles (of 128 anchors) per core.
    k: number of positive candidate columns (boundary between the pos
       range [0,k) and the sign-flipped neg range [k,B)).
    """
    import concourse.bacc as bacc
    import concourse.tile as tile
    from concourse import mybir

    f32 = mybir.dt.float32
    bf16 = mybir.dt.bfloat16
    fp8 = mybir.dt.float8e4
    AOT = mybir.AluOpType

    SH = n_it * 128

    nc = bacc.Bacc(None)
    e1t = nc.declare_dram_parameter("e1t", [D, SH], fp8, isOutput=False)
    e2t = nc.declare_dram_parameter("e2t", [D, B], fp8, isOutput=False)
    tails = nc.declare_dram_parameter("tails", [8, SH + B], bf16, isOutput=False)
    outp = nc.declare_dram_parameter("out", [128, 2 * n_it], f32, isOutput=True)

    # per-group reduction segments: (lo, hi, is_pos) in global column coords
    def group_segs(g):
        glo, ghi = g * GW, (g + 1) * GW
        segs = []
        if glo < k:
            segs.append((glo, min(ghi, k), True))
        if ghi > k:
            segs.append((max(glo, k), ghi, False))
        return segs

    def even_chunks(w):
        """Split [0,w) into even-width chunks <= ACCW; odd leftover col
        is returned separately (handled by the direct f32 path)."""
        out = []
        pos = 0
        we = (w // 2) * 2
        while pos < we:
            cw = min(ACCW, we - pos)
            out.append((pos, cw))
            pos += cw
        return out, (w - we)   # chunks, n leftover cols (0 or 1)

    with tile.TileContext(nc) as tc:
        bigneg16 = int(np.array(-BIG, ml_dtypes.bfloat16).view(np.uint16))
        bigneg32 = (bigneg16 << 16) | bigneg16

        with (
            tc.tile_pool(name="const", bufs=1) as cpool,
            tc.tile_pool(name="e2p", bufs=NG) as e2pool,
            tc.tile_pool(name="ps", bufs=PSUM_BUFS, space="PSUM") as pspool,
            tc.tile_pool(name="work", bufs=16) as workpool,
        ):
            cppool = workpool
            accpool = workpool
            redpool = workpool
            # merged bias-tail operands: strip s on partitions 32s..32s+1,
            # cols [0:SH] = lhsT (ones), [SH:] = rhs (bias hi/lo).  Split
            # each strip across both HWDGE queues: the destination spans
            # only 2 partitions, so per-partition write rate is the wall.
            e1sb = cpool.tile([D, SH], fp8, tag="e1sb")
            nc.sync.dma_start(e1sb[:], e1t[:])
            tlsb = cpool.tile([128, SH + B], bf16, tag="tlsb")
            q = (SH + B) // 4
            for s in range(4):
                for j in range(4):
                    eng = nc.sync if j % 2 == 0 else nc.scalar
                    lo, hi = j * q, (j + 1) * q if j < 3 else (SH + B)
                    hi = (j + 1) * q if j < 3 else (SH + B)
                    eng.dma_start(
                        tlsb[32 * s:32 * s + 2, lo:hi],
                        tails[2 * s:2 * s + 2, lo:hi],
                    )
            outsb = cpool.tile([128, 2 * n_it], f32, tag="outsb")
            trf = cpool.tile([128, 2048], f32, tag="trf")
            negc = cpool.tile([128, 2 * ACCW], bf16, tag="negc")
            nc.vector.memset(negc[:].bitcast(mybir.dt.uint32), bigneg32)

            e2sb = []
            for g in range(NG):
                e2c = e2pool.tile([D, GW], fp8, tag="e2c")
                nc.scalar.dma_start(e2c[:], e2t[:, g * GW:(g + 1) * GW])
                e2sb.append(e2c)

            pending_final = [None]

            def flush_final():
                if pending_final[0] is not None:
                    pending_final[0]()
                    pending_final[0] = None

            for it in range(n_it):
                icols = slice(it * 128, (it + 1) * 128)
                w8 = e1sb[:, icols]
                # one accumulator tile per i-tile: cols [0:ACCW] pos,
                # [ACCW:2*ACCW] neg -> single memset, single merged final
                acc2 = accpool.tile([128, 2 * ACCW], bf16, tag="acc",
                                    name=f"acc_{it}")
                nc.vector.tensor_copy(
                    acc2[:].bitcast(mybir.dt.uint32),
                    negc[:].bitcast(mybir.dt.uint32),
                )
                chaincol = redpool.tile([128, 2], f32, tag="chaincol",
                                        name=f"chain_{it}")
                chainb = redpool.tile([128, 2], bf16, tag="chainb",
                                      name=f"chainb_{it}")
                chain_used = {True: False, False: False}

                def drain_group(g, ps, chain_used=chain_used, acc2=acc2,
                                chaincol=chaincol, it=it):
                    for lo, hi, is_pos in group_segs(g):
                        ll, lh = lo - g * GW, hi - g * GW
                        wseg = lh - ll
                        ci = 0 if is_pos else 1
                        a0 = 0 if is_pos else ACCW
                        if wseg < MIN_COPY:
                            # direct f32 chained reduce of the sliver
                            nc.vector.tensor_scalar(
                                out=trf[:, 0:wseg],
                                in0=ps[:, ll:lh],
                                scalar1=(chaincol[:, ci:ci + 1]
                                         if chain_used[is_pos] else -BIG),
                                scalar2=None,
                                op0=AOT.max,
                                op1=AOT.max,
                                accum_out=chaincol[:, ci:ci + 1],
                            )
                            chain_used[is_pos] = True
                            continue
                        chunks, leftover = even_chunks(wseg)
                        cpb = cppool.tile([128, 2048], bf16, tag="cpb",
                                          name=f"cpb_{it}_{g}_{int(is_pos)}")
                        we = wseg - leftover
                        nc.scalar.copy(cpb[:, 0:we], ps[:, ll:ll + we])
                        for (cpos, cw) in chunks:
                            nc.vector.tensor_tensor(
                                acc2[:, a0:a0 + cw],
                                acc2[:, a0:a0 + cw],
                                cpb[:, cpos:cpos + cw],
                                op=AOT.max,
                            )
                        if leftover:
                            nc.vector.tensor_scalar(
                                out=trf[:, 0:leftover],
                                in0=ps[:, lh - leftover:lh],
                                scalar1=(chaincol[:, ci:ci + 1]
                                         if chain_used[is_pos] else -BIG),
                                scalar2=None,
                                op0=AOT.max,
                                op1=AOT.max,
                                accum_out=chaincol[:, ci:ci + 1],
                            )
                            chain_used[is_pos] = True

                pairs = [(0,), (1,), (2, 3)] if it == 0 else [(0, 1), (2, 3)]
                for pi, pair in enumerate(pairs):
                    pss = {}
                    for g in pair:
                        pss[g] = pspool.tile([128, GW], f32, tag="ps", name=f"ps_{it}_{g}")
                    # Steady state: bias tails FIRST (start=True), fp8 mains
                    # LAST (stop=True) so a group's drain unblocks right
                    # after its mains under the scheduler's weight batching.
                    # Ramp (first two groups of it 0): mains FIRST -- they
                    # only need the fast all-partition e1/e2 DMAs, while the
                    # tails wait on the slow 2-partition bias-strip DMAs.
                    mains_first = False

                    def emit_tails(g, start):
                        for s in range(GW // 512):
                            j0 = SH + g * GW + s * 512
                            nc.tensor.matmul(
                                pss[g][:, s * 512:(s + 1) * 512],
                                tlsb[32 * s:32 * s + 2, icols],
                                tlsb[32 * s:32 * s + 2, j0:j0 + 512],
                                start=start,
                                stop=not start,
                                tile_position=(32 * s, 0),
                            )

                    def emit_mains(g, start):
                        for s in range(GW // 512):
                            nc.tensor.matmul(
                                pss[g][:, s * 512:(s + 1) * 512],
                                w8,
                                e2sb[g][:, s * 512:(s + 1) * 512],
                                start=start,
                                stop=not start,
                            )

                    if mains_first:
                        for g in pair:
                            emit_mains(g, True)
                        for g in pair:
                            emit_tails(g, False)
                            drain_group(g, pss[g])
                    else:
                        for g in pair:
                            emit_tails(g, True)
                        for g in pair:
                            emit_mains(g, False)
                            drain_group(g, pss[g])
                    if pi == 0:
                        # previous i-tile's final runs here, off the
                        # PSUM-freeing critical path
                        flush_final()

                def make_final(it=it, acc2=acc2, chaincol=chaincol,
                               chainb=chainb, cu=chain_used):
                    def emit():
                        # fold chained slivers into the accumulator, then
                        # one 2-output max-reduce into the output columns
                        for is_pos in (True, False):
                            if cu[is_pos]:
                                ci = 0 if is_pos else 1
                                a0 = 0 if is_pos else ACCW
                                nc.vector.tensor_copy(
                                    chainb[:, ci:ci + 1], chaincol[:, ci:ci + 1]
                                )
                                nc.vector.tensor_tensor(
                                    acc2[:, a0:a0 + 1],
                                    acc2[:, a0:a0 + 1],
                                    chainb[:, ci:ci + 1],
                                    op=AOT.max,
                                )
                        nc.vector.tensor_reduce(
                            outsb[:, 2 * it:2 * it + 2],
                            acc2[:].rearrange("p (c w) -> p c w", c=2),
                            axis=mybir.AxisListType.X,
                            op=AOT.max,
                        )
                    return emit

                pending_final[0] = make_final()
            flush_final()
            nc.sync.dma_start(outp[:], outsb[:])
    nc.compile()
    return nc


def _host_prep(emb1, emb2, target):
    tpos = target == 1
    k = int(tpos.sum())
    perm = np.concatenate([np.nonzero(tpos)[0], np.nonzero(~tpos)[0]])
    e2s = emb2[perm]
    e2d = e2s.astype(np.float64)
    b = (e2d * e2d).sum(1) - (2.0 * EPS) * e2d.sum(1)
    sgn = np.ones(B, dtype=np.float64)
    sgn[k:] = -1.0
    bsig = (b * sgn).astype(np.float32)
    e2sig = e2s * sgn[:, None].astype(np.float32)

    e1p = emb1[tpos]                       # [k, D] pos anchors
    e1d = e1p.astype(np.float64)
    a = (e1d * e1d).sum(1) + (2.0 * EPS) * e1d.sum(1) + D * EPS * EPS

    n_it = min(k // 1024, 8)
    ndev = n_it * 1024
    e1dev = e1p[:ndev]

    e1m2t = np.ascontiguousarray((-2.0 * e1dev).T)          # [D, ndev] f32
    e2sigt = np.ascontiguousarray(e2sig.T)                  # [D, B] f32
    e1t8 = e1m2t.astype(ml_dtypes.float8_e4m3)
    e2t8 = e2sigt.astype(ml_dtypes.float8_e4m3)
    bhi = bsig.astype(ml_dtypes.bfloat16)
    blo = (bsig - bhi.astype(np.float32)).astype(ml_dtypes.bfloat16)
    # K=2 bias-tail operands; on device row 2s+r lands at partition 32s+r
    # so the four 512-wide sub-tiles of a group row-pack on the PE array.
    # Per-core layout: cols [0:SH] = lhsT (ones), [SH:] = rhs (bias).
    SH = n_it * 128
    trhs = np.zeros((8, B), dtype=ml_dtypes.bfloat16)
    for s in range(4):
        trhs[2 * s + 0] = bhi
        trhs[2 * s + 1] = blo
    onesw = np.ones((8, SH), dtype=ml_dtypes.bfloat16)
    tails = np.concatenate([onesw, trhs], axis=1)
    return k, n_it, a, e1p, e1t8, e2t8, tails


def _host_remainder(e1rem, emb2, target):
    """Exact f64 pos_max/neg_min contribution of the remainder anchors."""
    e1d = e1rem.astype(np.float64)
    e2d = emb2.astype(np.float64)
    sq = (
        (e1d * e1d).sum(1)[:, None]
        + (e2d * e2d).sum(1)[None, :]
        - 2.0 * (e1d @ e2d.T)
        + 2.0 * EPS * (e1d.sum(1)[:, None] - e2d.sum(1)[None, :])
        + D * EPS * EPS
    )
    dist = np.sqrt(np.clip(sq, 0.0, None))
    pos = target == 1
    pos_max = np.where(pos[None, :], dist, -np.inf).max(1)
    neg_min = np.where(~pos[None, :], dist, np.inf).min(1)
    return np.clip(pos_max - neg_min + MARGIN, 0.0, None).sum()


def _numpy_fallback(emb1, emb2, target):
    # exact reference recomputation in numpy (degenerate target mixes)
    e1 = emb1.astype(np.float64)
    e2 = emb2.astype(np.float64)
    sq = (
        (e1 * e1).sum(1)[:, None]
        + (e2 * e2).sum(1)[None, :]
        - 2.0 * (e1 @ e2.T)
        + 2.0 * EPS * (e1.sum(1)[:, None] - e2.sum(1)[None, :])
        + D * EPS * EPS
    )
    dist = np.sqrt(np.clip(sq, 0.0, None))
    pos = target == 1
    neg = target == 0
    pos_max = np.where(pos[None, :], dist, -np.inf).max(1)
    neg_min = np.where(neg[None, :], dist, np.inf).min(1)
    per = np.maximum(pos_max - neg_min + MARGIN, 0.0)
    w = pos.astype(np.float64)
    return np.float32((per * w).sum() / w.sum())


def kernel(emb1, emb2, target):
    global LAST_RESULTS
    emb1 = np.asarray(emb1, dtype=np.float32)
    emb2 = np.asarray(emb2, dtype=np.float32)
    target = np.asarray(target)
    assert emb1.shape == (B, D) and emb2.shape == (B, D)

    k = int((target == 1).sum())
    if k < 1024 or k == B:
        return _numpy_fallback(emb1, emb2, target)

    k, n_it, a, e1p, e1t8, e2t8, tails = _host_prep(emb1, emb2, target)
    ndev = n_it * 1024
    SH = n_it * 128

    nc = _programs.get((n_it, k))
    if nc is None:
        nc = _build_program(n_it, k)
        _programs[(n_it, k)] = nc

    from concourse.bass_utils import run_bass_kernel_spmd

    in_maps = [
        {
            "e1t": np.ascontiguousarray(e1t8[:, c * SH:(c + 1) * SH]),
            "e2t": e2t8,
            "tails": tails,
        }
        for c in range(NCORES)
    ]
    res = run_bass_kernel_spmd(nc, in_maps, core_ids=list(range(NCORES)))
    LAST_RESULTS = res

    Mp = np.concatenate(
        [np.asarray(res.results[c]["out"])[:, 0::2].T.reshape(-1) for c in range(NCORES)]
    )
    Mn = np.concatenate(
        [np.asarray(res.results[c]["out"])[:, 1::2].T.reshape(-1) for c in range(NCORES)]
    )

    adev = a[:ndev]
    pos2 = np.clip(adev + Mp.astype(np.float64), 0.0, None)
    neg2 = np.clip(adev - Mn.astype(np.float64), 0.0, None)  # min v = -max(-v)
    per = np.clip(np.sqrt(pos2) - np.sqrt(neg2) + MARGIN, 0.0, None)
    total = per.sum()
    if ndev < k:
        total += _host_remainder(e1p[ndev:], emb2, target)
    return np.float32(total / k)


# revision 32
# speedup vs baseline: 1.0215x; 1.0215x over previous
"""BatchHardTripletLoss kernel for 8 Trainium2 NeuronCores.

Math (matches the jax reference):
  dist2[i,j] = |e1_i|^2 + |e2_j|^2 - 2 e1.e2 + 2*eps*(s1_i - s2_j) + D*eps^2
             = a[i] + v[i,j],   v[i,j] = b[j] - 2<e1_i, e2_j>
  pos_max[i] = sqrt(clip(a[i] + max_{j in pos} v[i,j], 0))
  neg_min[i] = sqrt(clip(a[i] + min_{j in neg} v[i,j], 0))
  loss = mean over POS anchors of relu(pos_max - neg_min + margin)

Key structural points (vs the 106.7us first-cut kernel):
  * Only rows with target[i]==1 are computed (the loss ignores the
    rest): k//1024*1024 rows on device, the remainder exactly on host.
  * Neg-class e2 columns (and their bias) are sign-flipped so both
    classes are MAX reductions.
  * Mains run in fp8 (e4m3, K=128, all dims): the PE streams 2
    columns/cycle, so a 512-col matmul issues every ~216ns even at
    the cold (K=4/8 HAM) clock.  K=2 bf16 tails add the bias in
    bf16 hi+lo.  Verified end-to-end fp8 rel err ~6.8e-4 (tol 2e-2).
  * The bias TAILS run FIRST (start=True) and the mains LAST: the
    drain of a group unblocks as soon as its mains finish, which
    overlaps the scheduler's weight-batched [tails A,B][mains A,B]
    PE order instead of serializing PE-phase -> drain-phase.
  * Drain: the Act engine copies each PSUM group to SBUF bf16
    (1 elem/cyc); the DVE folds the copies in-place into a
    [128,1024] bf16 accumulator per (i-tile, class) with stock
    TENSOR_TENSOR max ops at the packed 2x rate, then one 1x
    TENSOR_SCALAR accumulate-max per (i-tile, class) reduces the
    accumulator into the output, chained with any direct-reduced
    boundary slivers via its per-partition scalar operand.

Host: pos-first column sort, f64 row stats, bf16 hi/lo bias split,
transposes, fp8/bf16 casts, final sqrt/margin/mean + exact f64
remainder rows.
"""

import os
import sys

for _p in ("/opt/trn_rl_repo",):
    if _p not in sys.path:
        sys.path.insert(0, _p)

import numpy as np
import ml_dtypes

EPS = 1e-6
MARGIN = 0.2
B = 8192
D = 128
NCORES = 8
GW = 2048             # candidate-group width = 4 PSUM banks
NG = B // GW
PSUM_BUFS = 2
BIG = 1.0e30
MIN_COPY = 512        # segments narrower than this reduce directly from PSUM
ACCW = 1024           # bf16 accumulator width per (i-tile, class)

_programs = {}
LAST_RESULTS = None   # BassKernelResults of the most recent run (for profiling)


def _build_program(n_it: int, k: int):
    """Bass program for one core.

    n_it: i-tiles (of 128 anchors) per core.
    k: number of positive candidate columns (boundary between the pos
       range [0,k) and the sign-flipped neg range [k,B)).
    """
    import concourse.bacc as bacc
    import concourse.tile as tile
    from concourse import mybir

    f32 = mybir.dt.float32
    bf16 = mybir.dt.bfloat16
    fp8 = mybir.dt.float8e4
    AOT = mybir.AluOpType

    SH = n_it * 128

    nc = bacc.Bacc(None)
    e1t = nc.declare_dram_parameter("e1t", [D, SH], fp8, isOutput=False)
    e2t = nc.declare_dram_parameter("e2t", [D, B], fp8, isOutput=False)
    tails = nc.declare_dram_parameter("tails", [8, SH + B], bf16, isOutput=False)
    outp = nc.declare_dram_parameter("out", [128, 2 * n_it], f32, isOutput=True)

    # per-group reduction segments: (lo, hi, is_pos) in global column coords
    def group_segs(g):
        glo, ghi = g * GW, (g + 1) * GW
        segs = []
        if glo < k:
            segs.append((glo, min(ghi, k), True))
        if ghi > k:
            segs.append((max(glo, k), ghi, False))
        return segs

    def even_chunks(w):
        """Split [0,w) into even-width chunks <= ACCW; odd leftover col
        is returned separately (handled by the direct f32 path)."""
        out = []
        pos = 0
        we = (w // 2) * 2
        while pos < we:
            cw = min(ACCW, we - pos)
            out.append((pos, cw))
            pos += cw
        return out, (w - we)   # chunks, n leftover cols (0 or 1)

    with tile.TileContext(nc) as tc:
        bigneg16 = int(np.array(-BIG, ml_dtypes.bfloat16).view(np.uint16))
        bigneg32 = (bigneg16 << 16) | bigneg16

        with (
            tc.tile_pool(name="const", bufs=1) as cpool,
            tc.tile_pool(name="e2p", bufs=NG) as e2pool,
            tc.tile_pool(name="ps", bufs=PSUM_BUFS, space="PSUM") as pspool,
            tc.tile_pool(name="work", bufs=16) as workpool,
        ):
            cppool = workpool
            accpool = workpool
            redpool = workpool
            # merged bias-tail operands: strip s on partitions 32s..32s+1,
            # cols [0:SH] = lhsT (ones), [SH:] = rhs (bias hi/lo).  Split
            # each strip across both HWDGE queues: the destination spans
            # only 2 partitions, so per-partition write rate is the wall.
            tlsb = cpool.tile([128, SH + B], bf16, tag="tlsb")
            half = (SH + B) // 2
            for s in range(4):
                nc.sync.dma_start(
                    tlsb[32 * s:32 * s + 2, 0:half], tails[2 * s:2 * s + 2, 0:half]
                )
                nc.scalar.dma_start(
                    tlsb[32 * s:32 * s + 2, half:], tails[2 * s:2 * s + 2, half:]
                )
            e1sb = cpool.tile([D, SH], fp8, tag="e1sb")
            nc.sync.dma_start(e1sb[:], e1t[:])
            outsb = cpool.tile([128, 2 * n_it], f32, tag="outsb")
            trf = cpool.tile([128, 2048], f32, tag="trf")
            negc = cpool.tile([128, 2 * ACCW], bf16, tag="negc")
            nc.vector.memset(negc[:].bitcast(mybir.dt.uint32), bigneg32)

            e2sb = []
            for g in range(NG):
                e2c = e2pool.tile([D, GW], fp8, tag="e2c")
                nc.scalar.dma_start(e2c[:], e2t[:, g * GW:(g + 1) * GW])
                e2sb.append(e2c)

            pending_final = [None]

            def flush_final():
                if pending_final[0] is not None:
                    pending_final[0]()
                    pending_final[0] = None

            for it in range(n_it):
                icols = slice(it * 128, (it + 1) * 128)
                w8 = e1sb[:, icols]
                # one accumulator tile per i-tile: cols [0:ACCW] pos,
                # [ACCW:2*ACCW] neg -> single memset, single merged final
                acc2 = accpool.tile([128, 2 * ACCW], bf16, tag="acc",
                                    name=f"acc_{it}")
                nc.vector.tensor_copy(
                    acc2[:].bitcast(mybir.dt.uint32),
                    negc[:].bitcast(mybir.dt.uint32),
                )
                chaincol = redpool.tile([128, 2], f32, tag="chaincol",
                                        name=f"chain_{it}")
                chainb = redpool.tile([128, 2], bf16, tag="chainb",
                                      name=f"chainb_{it}")
                chain_used = {True: False, False: False}

                def drain_group(g, ps, chain_used=chain_used, acc2=acc2,
                                chaincol=chaincol, it=it):
                    for lo, hi, is_pos in group_segs(g):
                        ll, lh = lo - g * GW, hi - g * GW
                        wseg = lh - ll
                        ci = 0 if is_pos else 1
                        a0 = 0 if is_pos else ACCW
                        if wseg < MIN_COPY:
                            # direct f32 chained reduce of the sliver
                            nc.vector.tensor_scalar(
                                out=trf[:, 0:wseg],
                                in0=ps[:, ll:lh],
                                scalar1=(chaincol[:, ci:ci + 1]
                                         if chain_used[is_pos] else -BIG),
                                scalar2=None,
                                op0=AOT.max,
                                op1=AOT.max,
                                accum_out=chaincol[:, ci:ci + 1],
                            )
                            chain_used[is_pos] = True
                            continue
                        chunks, leftover = even_chunks(wseg)
                        cpb = cppool.tile([128, 2048], bf16, tag="cpb",
                                          name=f"cpb_{it}_{g}_{int(is_pos)}")
                        we = wseg - leftover
                        nc.scalar.copy(cpb[:, 0:we], ps[:, ll:ll + we])
                        for (cpos, cw) in chunks:
                            nc.vector.tensor_tensor(
                                acc2[:, a0:a0 + cw],
                                acc2[:, a0:a0 + cw],
                                cpb[:, cpos:cpos + cw],
                                op=AOT.max,
                            )
                        if leftover:
                            nc.vector.tensor_scalar(
                                out=trf[:, 0:leftover],
                                in0=ps[:, lh - leftover:lh],
                                scalar1=(chaincol[:, ci:ci + 1]
                                         if chain_used[is_pos] else -BIG),
                                scalar2=None,
                                op0=AOT.max,
                                op1=AOT.max,
                                accum_out=chaincol[:, ci:ci + 1],
                            )
                            chain_used[is_pos] = True

                pairs = [(0,), (1,), (2, 3)] if it == 0 else [(0, 1), (2, 3)]
                for pi, pair in enumerate(pairs):
                    pss = {}
                    for g in pair:
                        pss[g] = pspool.tile([128, GW], f32, tag="ps", name=f"ps_{it}_{g}")
                    # Steady state: bias tails FIRST (start=True), fp8 mains
                    # LAST (stop=True) so a group's drain unblocks right
                    # after its mains under the scheduler's weight batching.
                    # Ramp (first two groups of it 0): mains FIRST -- they
                    # only need the fast all-partition e1/e2 DMAs, while the
                    # tails wait on the slow 2-partition bias-strip DMAs.
                    mains_first = False

                    def emit_tails(g, start):
                        for s in range(GW // 512):
                            j0 = SH + g * GW + s * 512
                            nc.tensor.matmul(
                                pss[g][:, s * 512:(s + 1) * 512],
                                tlsb[32 * s:32 * s + 2, icols],
                                tlsb[32 * s:32 * s + 2, j0:j0 + 512],
                                start=start,
                                stop=not start,
                                tile_position=(32 * s, 0),
                            )

                    def emit_mains(g, start):
                        for s in range(GW // 512):
                            nc.tensor.matmul(
                                pss[g][:, s * 512:(s + 1) * 512],
                                w8,
                                e2sb[g][:, s * 512:(s + 1) * 512],
                                start=start,
                                stop=not start,
                            )

                    if mains_first:
                        for g in pair:
                            emit_mains(g, True)
                        for g in pair:
                            emit_tails(g, False)
                            drain_group(g, pss[g])
                    else:
                        for g in pair:
                            emit_tails(g, True)
                        for g in pair:
                            emit_mains(g, False)
                            drain_group(g, pss[g])
                    if pi == 0:
                        # previous i-tile's final runs here, off the
                        # PSUM-freeing critical path
                        flush_final()

                def make_final(it=it, acc2=acc2, chaincol=chaincol,
                               chainb=chainb, cu=chain_used):
                    def emit():
                        # fold chained slivers into the accumulator, then
                        # one 2-output max-reduce into the output columns
                        for is_pos in (True, False):
                            if cu[is_pos]:
                                ci = 0 if is_pos else 1
                                a0 = 0 if is_pos else ACCW
                                nc.vector.tensor_copy(
                                    chainb[:, ci:ci + 1], chaincol[:, ci:ci + 1]
                                )
                                nc.vector.tensor_tensor(
                                    acc2[:, a0:a0 + 1],
                                    acc2[:, a0:a0 + 1],
                                    chainb[:, ci:ci + 1],
                                    op=AOT.max,
                                )
                        nc.vector.tensor_reduce(
                            outsb[:, 2 * it:2 * it + 2],
                            acc2[:].rearrange("p (c w) -> p c w", c=2),
                            axis=mybir.AxisListType.X,
                            op=AOT.max,
                        )
                    return emit

                pending_final[0] = make_final()
            flush_final()
            nc.sync.dma_start(outp[:], outsb[:])
    nc.compile()
    return nc


def _host_prep(emb1, emb2, target):
    tpos = target == 1
    k = int(tpos.sum())
    perm = np.concatenate([np.nonzero(tpos)[0], np.nonzero(~tpos)[0]])
    e2s = emb2[perm]
    e2d = e2s.astype(np.float64)
    b = (e2d * e2d).sum(1) - (2.0 * EPS) * e2d.sum(1)
    sgn = np.ones(B, dtype=np.float64)
    sgn[k:] = -1.0
    bsig = (b * sgn).astype(np.float32)
    e2sig = e2s * sgn[:, None].astype(np.float32)

    e1p = emb1[tpos]                       # [k, D] pos anchors
    e1d = e1p.astype(np.float64)
    a = (e1d * e1d).sum(1) + (2.0 * EPS) * e1d.sum(1) + D * EPS * EPS

    n_it = min(k // 1024, 8)
    ndev = n_it * 1024
    e1dev = e1p[:ndev]

    e1m2t = np.ascontiguousarray((-2.0 * e1dev).T)          # [D, ndev] f32
    e2sigt = np.ascontiguousarray(e2sig.T)                  # [D, B] f32
    e1t8 = e1m2t.astype(ml_dtypes.float8_e4m3)
    e2t8 = e2sigt.astype(ml_dtypes.float8_e4m3)
    bhi = bsig.astype(ml_dtypes.bfloat16)
    blo = (bsig - bhi.astype(np.float32)).astype(ml_dtypes.bfloat16)
    # K=2 bias-tail operands; on device row 2s+r lands at partition 32s+r
    # so the four 512-wide sub-tiles of a group row-pack on the PE array.
    # Per-core layout: cols [0:SH] = lhsT (ones), [SH:] = rhs (bias).
    SH = n_it * 128
    trhs = np.zeros((8, B), dtype=ml_dtypes.bfloat16)
    for s in range(4):
        trhs[2 * s + 0] = bhi
        trhs[2 * s + 1] = blo
    onesw = np.ones((8, SH), dtype=ml_dtypes.bfloat16)
    tails = np.concatenate([onesw, trhs], axis=1)
    return k, n_it, a, e1p, e1t8, e2t8, tails


def _host_remainder(e1rem, emb2, target):
    """Exact f64 pos_max/neg_min contribution of the remainder anchors."""
    e1d = e1rem.astype(np.float64)
    e2d = emb2.astype(np.float64)
    sq = (
        (e1d * e1d).sum(1)[:, None]
        + (e2d * e2d).sum(1)[None, :]
        - 2.0 * (e1d @ e2d.T)
        + 2.0 * EPS * (e1d.sum(1)[:, None] - e2d.sum(1)[None, :])
        + D * EPS * EPS
    )
    dist = np.sqrt(np.clip(sq, 0.0, None))
    pos = target == 1
    pos_max = np.where(pos[None, :], dist, -np.inf).max(1)
    neg_min = np.where(~pos[None, :], dist, np.inf).min(1)
    return np.clip(pos_max - neg_min + MARGIN, 0.0, None).sum()


def _numpy_fallback(emb1, emb2, target):
    # exact reference recomputation in numpy (degenerate target mixes)
    e1 = emb1.astype(np.float64)
    e2 = emb2.astype(np.float64)
    sq = (
        (e1 * e1).sum(1)[:, None]
        + (e2 * e2).sum(1)[None, :]
        - 2.0 * (e1 @ e2.T)
        + 2.0 * EPS * (e1.sum(1)[:, None] - e2.sum(1)[None, :])
        + D * EPS * EPS
    )
    dist = np.sqrt(np.clip(sq, 0.0, None))
    pos = target == 1
    neg = target == 0
    pos_max = np.where(pos[None, :], dist, -np.inf).max(1)
    neg_min = np.where(neg[None, :], dist, np.inf).min(1)
    per = np.maximum(pos_max - neg_min + MARGIN, 0.0)
    w = pos.astype(np.float64)
    return np.float32((per * w).sum() / w.sum())


def kernel(emb1, emb2, target):
    global LAST_RESULTS
    emb1 = np.asarray(emb1, dtype=np.float32)
    emb2 = np.asarray(emb2, dtype=np.float32)
    target = np.asarray(target)
    assert emb1.shape == (B, D) and emb2.shape == (B, D)

    k = int((target == 1).sum())
    if k < 1024 or k == B:
        return _numpy_fallback(emb1, emb2, target)

    k, n_it, a, e1p, e1t8, e2t8, tails = _host_prep(emb1, emb2, target)
    ndev = n_it * 1024
    SH = n_it * 128

    nc = _programs.get((n_it, k))
    if nc is None:
        nc = _build_program(n_it, k)
        _programs[(n_it, k)] = nc

    from concourse.bass_utils import run_bass_kernel_spmd

    in_maps = [
        {
            "e1t": np.ascontiguousarray(e1t8[:, c * SH:(c + 1) * SH]),
            "e2t": e2t8,
            "tails": tails,
        }
        for c in range(NCORES)
    ]
    res = run_bass_kernel_spmd(nc, in_maps, core_ids=list(range(NCORES)))
    LAST_RESULTS = res

    Mp = np.concatenate(
        [np.asarray(res.results[c]["out"])[:, 0::2].T.reshape(-1) for c in range(NCORES)]
    )
    Mn = np.concatenate(
        [np.asarray(res.results[c]["out"])[:, 1::2].T.reshape(-1) for c in range(NCORES)]
    )

    adev = a[:ndev]
    pos2 = np.clip(adev + Mp.astype(np.float64), 0.0, None)
    neg2 = np.clip(adev - Mn.astype(np.float64), 0.0, None)  # min v = -max(-v)
    per = np.clip(np.sqrt(pos2) - np.sqrt(neg2) + MARGIN, 0.0, None)
    total = per.sum()
    if ndev < k:
        total += _host_remainder(e1p[ndev:], emb2, target)
    return np.float32(total / k)


# revision 33
# speedup vs baseline: 1.0461x; 1.0241x over previous
"""BatchHardTripletLoss kernel for 8 Trainium2 NeuronCores.

Math (matches the jax reference):
  dist2[i,j] = |e1_i|^2 + |e2_j|^2 - 2 e1.e2 + 2*eps*(s1_i - s2_j) + D*eps^2
             = a[i] + v[i,j],   v[i,j] = b[j] - 2<e1_i, e2_j>
  pos_max[i] = sqrt(clip(a[i] + max_{j in pos} v[i,j], 0))
  neg_min[i] = sqrt(clip(a[i] + min_{j in neg} v[i,j], 0))
  loss = mean over POS anchors of relu(pos_max - neg_min + margin)

Key structural points (vs the 106.7us first-cut kernel):
  * Only rows with target[i]==1 are computed (the loss ignores the
    rest): k//1024*1024 rows on device, the remainder exactly on host.
  * Neg-class e2 columns (and their bias) are sign-flipped so both
    classes are MAX reductions.
  * Mains run in fp8 (e4m3, K=128, all dims): the PE streams 2
    columns/cycle, so a 512-col matmul issues every ~216ns even at
    the cold (K=4/8 HAM) clock.  K=2 bf16 tails add the bias in
    bf16 hi+lo.  Verified end-to-end fp8 rel err ~6.8e-4 (tol 2e-2).
  * The bias TAILS run FIRST (start=True) and the mains LAST: the
    drain of a group unblocks as soon as its mains finish, which
    overlaps the scheduler's weight-batched [tails A,B][mains A,B]
    PE order instead of serializing PE-phase -> drain-phase.
  * Drain: the Act engine copies each PSUM group to SBUF bf16
    (1 elem/cyc); the DVE folds the copies in-place into a
    [128,1024] bf16 accumulator per (i-tile, class) with stock
    TENSOR_TENSOR max ops at the packed 2x rate, then one 1x
    TENSOR_SCALAR accumulate-max per (i-tile, class) reduces the
    accumulator into the output, chained with any direct-reduced
    boundary slivers via its per-partition scalar operand.

Host: pos-first column sort, f64 row stats, bf16 hi/lo bias split,
transposes, fp8/bf16 casts, final sqrt/margin/mean + exact f64
remainder rows.
"""

import os
import sys

for _p in ("/opt/trn_rl_repo",):
    if _p not in sys.path:
        sys.path.insert(0, _p)

import numpy as np
import ml_dtypes

EPS = 1e-6
MARGIN = 0.2
B = 8192
D = 128
NCORES = 8
GW = 2048             # candidate-group width = 4 PSUM banks
NG = B // GW
PSUM_BUFS = 2
BIG = 1.0e30
MIN_COPY = 512        # segments narrower than this reduce directly from PSUM
ACCW = 1024           # bf16 accumulator width per (i-tile, class)

_programs = {}
LAST_RESULTS = None   # BassKernelResults of the most recent run (for profiling)


def _build_program(n_it: int, k: int):
    """Bass program for one core.

    n_it: i-tiles (of 128 anchors) per core.
    k: number of positive candidate columns (boundary between the pos
       range [0,k) and the sign-flipped neg range [k,B)).
    """
    import concourse.bacc as bacc
    import concourse.tile as tile
    from concourse import mybir

    f32 = mybir.dt.float32
    bf16 = mybir.dt.bfloat16
    fp8 = mybir.dt.float8e4
    AOT = mybir.AluOpType

    SH = n_it * 128

    nc = bacc.Bacc(None)
    e1t = nc.declare_dram_parameter("e1t", [D, SH], fp8, isOutput=False)
    e2t = nc.declare_dram_parameter("e2t", [D, B], fp8, isOutput=False)
    tails = nc.declare_dram_parameter("tails", [8, SH + B], bf16, isOutput=False)
    outp = nc.declare_dram_parameter("out", [128, 2 * n_it], f32, isOutput=True)

    # per-group reduction segments: (lo, hi, is_pos) in global column coords
    def group_segs(g):
        glo, ghi = g * GW, (g + 1) * GW
        segs = []
        if glo < k:
            segs.append((glo, min(ghi, k), True))
        if ghi > k:
            segs.append((max(glo, k), ghi, False))
        return segs

    def even_chunks(w):
        """Split [0,w) into even-width chunks <= ACCW; odd leftover col
        is returned separately (handled by the direct f32 path)."""
        out = []
        pos = 0
        we = (w // 2) * 2
        while pos < we:
            cw = min(ACCW, we - pos)
            out.append((pos, cw))
            pos += cw
        return out, (w - we)   # chunks, n leftover cols (0 or 1)

    with tile.TileContext(nc) as tc:
        bigneg16 = int(np.array(-BIG, ml_dtypes.bfloat16).view(np.uint16))
        bigneg32 = (bigneg16 << 16) | bigneg16

        with (
            tc.tile_pool(name="const", bufs=1) as cpool,
            tc.tile_pool(name="e2p", bufs=NG) as e2pool,
            tc.tile_pool(name="ps", bufs=PSUM_BUFS, space="PSUM") as pspool,
            tc.tile_pool(name="work", bufs=16) as workpool,
        ):
            cppool = workpool
            accpool = workpool
            redpool = workpool
            # merged bias-tail operands: strip s on partitions 32s..32s+1,
            # cols [0:SH] = lhsT (ones), [SH:] = rhs (bias hi/lo).  Split
            # each strip across both HWDGE queues: the destination spans
            # only 2 partitions, so per-partition write rate is the wall.
            tlsb = cpool.tile([128, SH + B], bf16, tag="tlsb")
            half = (SH + B) // 2
            for s in range(4):
                nc.sync.dma_start(
                    tlsb[32 * s:32 * s + 2, 0:half], tails[2 * s:2 * s + 2, 0:half]
                )
                nc.scalar.dma_start(
                    tlsb[32 * s:32 * s + 2, half:], tails[2 * s:2 * s + 2, half:]
                )
            e1sb = cpool.tile([D, SH], fp8, tag="e1sb")
            nc.sync.dma_start(e1sb[:], e1t[:])
            outsb = cpool.tile([128, 2 * n_it], f32, tag="outsb")
            trf = cpool.tile([128, 2048], f32, tag="trf")
            negc = cpool.tile([128, 2 * ACCW], bf16, tag="negc")
            nc.vector.memset(negc[:].bitcast(mybir.dt.uint32), bigneg32)

            e2sb = []
            for g in range(NG):
                e2c = e2pool.tile([D, GW], fp8, tag="e2c")
                nc.scalar.dma_start(e2c[:], e2t[:, g * GW:(g + 1) * GW])
                e2sb.append(e2c)

            pending_final = [None]

            def flush_final():
                if pending_final[0] is not None:
                    pending_final[0]()
                    pending_final[0] = None

            for it in range(n_it):
                icols = slice(it * 128, (it + 1) * 128)
                w8 = e1sb[:, icols]
                # one accumulator tile per i-tile: cols [0:ACCW] pos,
                # [ACCW:2*ACCW] neg -> single memset, single merged final
                acc2 = accpool.tile([128, 2 * ACCW], bf16, tag="acc",
                                    name=f"acc_{it}")
                nc.gpsimd.memset(acc2[:], -BIG)
                chaincol = redpool.tile([128, 2], f32, tag="chaincol",
                                        name=f"chain_{it}")
                chainb = redpool.tile([128, 2], bf16, tag="chainb",
                                      name=f"chainb_{it}")
                chain_used = {True: False, False: False}

                def drain_group(g, ps, chain_used=chain_used, acc2=acc2,
                                chaincol=chaincol, it=it):
                    for lo, hi, is_pos in group_segs(g):
                        ll, lh = lo - g * GW, hi - g * GW
                        wseg = lh - ll
                        ci = 0 if is_pos else 1
                        a0 = 0 if is_pos else ACCW
                        if wseg < MIN_COPY:
                            # direct f32 chained reduce of the sliver
                            nc.vector.tensor_scalar(
                                out=trf[:, 0:wseg],
                                in0=ps[:, ll:lh],
                                scalar1=(chaincol[:, ci:ci + 1]
                                         if chain_used[is_pos] else -BIG),
                                scalar2=None,
                                op0=AOT.max,
                                op1=AOT.max,
                                accum_out=chaincol[:, ci:ci + 1],
                            )
                            chain_used[is_pos] = True
                            continue
                        chunks, leftover = even_chunks(wseg)
                        cpb = cppool.tile([128, 2048], bf16, tag="cpb",
                                          name=f"cpb_{it}_{g}_{int(is_pos)}")
                        we = wseg - leftover
                        nc.scalar.copy(cpb[:, 0:we], ps[:, ll:ll + we])
                        for (cpos, cw) in chunks:
                            nc.vector.tensor_tensor(
                                acc2[:, a0:a0 + cw],
                                acc2[:, a0:a0 + cw],
                                cpb[:, cpos:cpos + cw],
                                op=AOT.max,
                            )
                        if leftover:
                            nc.vector.tensor_scalar(
                                out=trf[:, 0:leftover],
                                in0=ps[:, lh - leftover:lh],
                                scalar1=(chaincol[:, ci:ci + 1]
                                         if chain_used[is_pos] else -BIG),
                                scalar2=None,
                                op0=AOT.max,
                                op1=AOT.max,
                                accum_out=chaincol[:, ci:ci + 1],
                            )
                            chain_used[is_pos] = True

                pairs = [(0,), (1,), (2, 3)] if it == 0 else [(0, 1), (2, 3)]
                for pi, pair in enumerate(pairs):
                    pss = {}
                    for g in pair:
                        pss[g] = pspool.tile([128, GW], f32, tag="ps", name=f"ps_{it}_{g}")
                    # Steady state: bias tails FIRST (start=True), fp8 mains
                    # LAST (stop=True) so a group's drain unblocks right
                    # after its mains under the scheduler's weight batching.
                    # Ramp (first two groups of it 0): mains FIRST -- they
                    # only need the fast all-partition e1/e2 DMAs, while the
                    # tails wait on the slow 2-partition bias-strip DMAs.
                    mains_first = False

                    def emit_tails(g, start):
                        for s in range(GW // 512):
                            j0 = SH + g * GW + s * 512
                            nc.tensor.matmul(
                                pss[g][:, s * 512:(s + 1) * 512],
                                tlsb[32 * s:32 * s + 2, icols],
                                tlsb[32 * s:32 * s + 2, j0:j0 + 512],
                                start=start,
                                stop=not start,
                                tile_position=(32 * s, 0),
                            )

                    def emit_mains(g, start):
                        for s in range(GW // 512):
                            nc.tensor.matmul(
                                pss[g][:, s * 512:(s + 1) * 512],
                                w8,
                                e2sb[g][:, s * 512:(s + 1) * 512],
                                start=start,
                                stop=not start,
                            )

                    if mains_first:
                        for g in pair:
                            emit_mains(g, True)
                        for g in pair:
                            emit_tails(g, False)
                            drain_group(g, pss[g])
                    else:
                        for g in pair:
                            emit_tails(g, True)
                        for g in pair:
                            emit_mains(g, False)
                            drain_group(g, pss[g])
                    if pi == 0:
                        # previous i-tile's final runs here, off the
                        # PSUM-freeing critical path
                        flush_final()

                def make_final(it=it, acc2=acc2, chaincol=chaincol,
                               chainb=chainb, cu=chain_used):
                    def emit():
                        # fold chained slivers into the accumulator, then
                        # one 2-output max-reduce into the output columns
                        for is_pos in (True, False):
                            if cu[is_pos]:
                                ci = 0 if is_pos else 1
                                a0 = 0 if is_pos else ACCW
                                nc.vector.tensor_copy(
                                    chainb[:, ci:ci + 1], chaincol[:, ci:ci + 1]
                                )
                                nc.vector.tensor_tensor(
                                    acc2[:, a0:a0 + 1],
                                    acc2[:, a0:a0 + 1],
                                    chainb[:, ci:ci + 1],
                                    op=AOT.max,
                                )
                        nc.vector.tensor_reduce(
                            outsb[:, 2 * it:2 * it + 2],
                            acc2[:].rearrange("p (c w) -> p c w", c=2),
                            axis=mybir.AxisListType.X,
                            op=AOT.max,
                        )
                    return emit

                pending_final[0] = make_final()
            flush_final()
            nc.sync.dma_start(outp[:], outsb[:])
    nc.compile()
    return nc


def _host_prep(emb1, emb2, target):
    tpos = target == 1
    k = int(tpos.sum())
    perm = np.concatenate([np.nonzero(tpos)[0], np.nonzero(~tpos)[0]])
    e2s = emb2[perm]
    e2d = e2s.astype(np.float64)
    b = (e2d * e2d).sum(1) - (2.0 * EPS) * e2d.sum(1)
    sgn = np.ones(B, dtype=np.float64)
    sgn[k:] = -1.0
    bsig = (b * sgn).astype(np.float32)
    e2sig = e2s * sgn[:, None].astype(np.float32)

    e1p = emb1[tpos]                       # [k, D] pos anchors
    e1d = e1p.astype(np.float64)
    a = (e1d * e1d).sum(1) + (2.0 * EPS) * e1d.sum(1) + D * EPS * EPS

    n_it = min(k // 1024, 8)
    ndev = n_it * 1024
    e1dev = e1p[:ndev]

    e1m2t = np.ascontiguousarray((-2.0 * e1dev).T)          # [D, ndev] f32
    e2sigt = np.ascontiguousarray(e2sig.T)                  # [D, B] f32
    e1t8 = e1m2t.astype(ml_dtypes.float8_e4m3)
    e2t8 = e2sigt.astype(ml_dtypes.float8_e4m3)
    bhi = bsig.astype(ml_dtypes.bfloat16)
    blo = (bsig - bhi.astype(np.float32)).astype(ml_dtypes.bfloat16)
    # K=2 bias-tail operands; on device row 2s+r lands at partition 32s+r
    # so the four 512-wide sub-tiles of a group row-pack on the PE array.
    # Per-core layout: cols [0:SH] = lhsT (ones), [SH:] = rhs (bias).
    SH = n_it * 128
    trhs = np.zeros((8, B), dtype=ml_dtypes.bfloat16)
    for s in range(4):
        trhs[2 * s + 0] = bhi
        trhs[2 * s + 1] = blo
    onesw = np.ones((8, SH), dtype=ml_dtypes.bfloat16)
    tails = np.concatenate([onesw, trhs], axis=1)
    return k, n_it, a, e1p, e1t8, e2t8, tails


def _host_remainder(e1rem, emb2, target):
    """Exact f64 pos_max/neg_min contribution of the remainder anchors."""
    e1d = e1rem.astype(np.float64)
    e2d = emb2.astype(np.float64)
    sq = (
        (e1d * e1d).sum(1)[:, None]
        + (e2d * e2d).sum(1)[None, :]
        - 2.0 * (e1d @ e2d.T)
        + 2.0 * EPS * (e1d.sum(1)[:, None] - e2d.sum(1)[None, :])
        + D * EPS * EPS
    )
    dist = np.sqrt(np.clip(sq, 0.0, None))
    pos = target == 1
    pos_max = np.where(pos[None, :], dist, -np.inf).max(1)
    neg_min = np.where(~pos[None, :], dist, np.inf).min(1)
    return np.clip(pos_max - neg_min + MARGIN, 0.0, None).sum()


def _numpy_fallback(emb1, emb2, target):
    # exact reference recomputation in numpy (degenerate target mixes)
    e1 = emb1.astype(np.float64)
    e2 = emb2.astype(np.float64)
    sq = (
        (e1 * e1).sum(1)[:, None]
        + (e2 * e2).sum(1)[None, :]
        - 2.0 * (e1 @ e2.T)
        + 2.0 * EPS * (e1.sum(1)[:, None] - e2.sum(1)[None, :])
        + D * EPS * EPS
    )
    dist = np.sqrt(np.clip(sq, 0.0, None))
    pos = target == 1
    neg = target == 0
    pos_max = np.where(pos[None, :], dist, -np.inf).max(1)
    neg_min = np.where(neg[None, :], dist, np.inf).min(1)
    per = np.maximum(pos_max - neg_min + MARGIN, 0.0)
    w = pos.astype(np.float64)
    return np.float32((per * w).sum() / w.sum())


def kernel(emb1, emb2, target):
    global LAST_RESULTS
    emb1 = np.asarray(emb1, dtype=np.float32)
    emb2 = np.asarray(emb2, dtype=np.float32)
    target = np.asarray(target)
    assert emb1.shape == (B, D) and emb2.shape == (B, D)

    k = int((target == 1).sum())
    if k < 1024 or k == B:
        return _numpy_fallback(emb1, emb2, target)

    k, n_it, a, e1p, e1t8, e2t8, tails = _host_prep(emb1, emb2, target)
    ndev = n_it * 1024
    SH = n_it * 128

    nc = _programs.get((n_it, k))
    if nc is None:
        nc = _build_program(n_it, k)
        _programs[(n_it, k)] = nc

    from concourse.bass_utils import run_bass_kernel_spmd

    in_maps = [
        {
            "e1t": np.ascontiguousarray(e1t8[:, c * SH:(c + 1) * SH]),
            "e2t": e2t8,
            "tails": tails,
        }
        for c in range(NCORES)
    ]
    res = run_bass_kernel_spmd(nc, in_maps, core_ids=list(range(NCORES)))
    LAST_RESULTS = res

    Mp = np.concatenate(
        [np.asarray(res.results[c]["out"])[:, 0::2].T.reshape(-1) for c in range(NCORES)]
    )
    Mn = np.concatenate(
        [np.asarray(res.results[c]["out"])[:, 1::2].T.reshape(-1) for c in range(NCORES)]
    )

    adev = a[:ndev]
    pos2 = np.clip(adev + Mp.astype(np.float64), 0.0, None)
    neg2 = np.clip(adev - Mn.astype(np.float64), 0.0, None)  # min v = -max(-v)
    per = np.clip(np.sqrt(pos2) - np.sqrt(neg2) + MARGIN, 0.0, None)
    total = per.sum()
    if ndev < k:
        total += _host_remainder(e1p[ndev:], emb2, target)
    return np.float32(total / k)
